# revision 2
# baseline (speedup 1.0000x reference)
"""Conv1d  x[32,256,4096] * W[512,256,9] + b  (stride 1, pad 4)  -> [32,512,4096]
on 8 TRN2 NeuronCores, data-parallel over the batch dim (4 batches/core).

Per core the conv is computed as PSUM-accumulated matmuls:
    out[o, p] = bias[o] + sum_{k-tile, tap} W_t[c, o].T-applied  @  xpad[c, p+t]
Each [128 outch x 512 pos] output tile accumulates 2 k-tiles x 9 taps = 18
matmuls.  The padded x rows ([128, 4104]) live whole in SBUF, so the rhs for
tap t is just an offset slice - no data movement between taps.  Weights are
pre-transposed on the host so each lhsT tile is a contiguous SBUF slice.
Matmuls run in float32r (full PE rate at N=512, ~TF32 precision).
"""

import os

import numpy as np

B, CIN, COUT, KW, L, PAD = 32, 256, 512, 9, 4096, 4
NCORES = 8
BPC = B // NCORES  # batches per core
LP = L + 2 * PAD  # padded length
KT = CIN // 128  # contraction k-tiles
MT = COUT // 128  # output-channel tiles
NFREE = 512  # matmul moving free dim
NT = L // NFREE  # output-position tiles

_CACHE = {}


def _split_multi_waits(nc, max_waits=1):
    # This container's walrus accepts at most one sync wait per instruction;
    # TileContext's tail drain carries several.  Hoist the excess onto
    # same-engine EventSemaphore instructions inserted just before it.
    import concourse.mybir as mybir

    for fn in nc.m.functions:
        for bb in fn.blocks:
            new_list = []
            changed = False
            for ins in bb.instructions:
                si = ins.sync_info
                if si is not None and si.on_wait and len(si.on_wait) > max_waits:
                    waits = list(si.on_wait)
                    hoist, keep = waits[:-max_waits], waits[-max_waits:]
                    for j, w in enumerate(hoist):
                        ev = mybir.InstEventSemaphore(
                            name=f"{ins.name}_wsplit{j}",
                            engine=ins.engine,
                            ins=[],
                            outs=[],
                            sync_info=mybir.SyncInfo(on_wait=[w], on_update=[]),
                        )
                        new_list.append(ev)
                    ins.sync_info = mybir.SyncInfo(
                        on_wait=keep, on_update=list(si.on_update)
                    )
                    changed = True
                new_list.append(ins)
            if changed:
                bb.instructions = new_list


def _build(reps=1, timing=False):
    """Build the per-core Bass module.

    timing=True makes the big I/O tensors Internal DRAM (nothing shipped
    through the axon tunnel) and wraps the whole body in a hardware For_i
    loop of `reps` iterations, so on-device time dominates wall clock."""
    import concourse.bass as bass
    import concourse.mybir as mybir
    import concourse.tile as tile

    f32 = mybir.dt.float32
    f32r = mybir.dt.float32r

    nc = bass.Bass()
    big_kind = "Internal" if timing else "ExternalInput"
    xp = nc.dram_tensor("xp", [BPC, CIN, LP], f32r, kind=big_kind)
    w = nc.dram_tensor("w", [KT, 128, KW, COUT], f32r, kind="ExternalInput")
    bias = nc.dram_tensor("bias", [128, MT], f32, kind="ExternalInput")
    out = nc.dram_tensor(
        "out", [BPC, COUT, L], f32, kind="Internal" if timing else "ExternalOutput"
    )
    done = (
        nc.dram_tensor("done", [1, 1], f32, kind="ExternalOutput") if timing else None
    )

    with tile.TileContext(nc) as tc:
        with (
            tc.tile_pool(name="wpool", bufs=1) as wpool,
            tc.tile_pool(name="xpool", bufs=2) as xpool,
            tc.tile_pool(name="opool", bufs=8) as opool,
            tc.tile_pool(name="psum", bufs=8, space="PSUM") as ppool,
        ):
            # One tile per (k-tile, tap): the first matmul only has to wait
            # for a 256 KB slice, not the whole 4.6 MB weight block (sim
            # showed a 21.7 us PE stall at startup with one big tile).
            # Tap-0 tiles are DMA'd first - they gate the first matmuls.
            wk = [[None] * KW for _ in range(KT)]
            for t in range(KW):
                for k in range(KT):
                    t_ = wpool.tile([128, COUT], f32r, name="wkt", tag=f"w{k}_{t}")
                    nc.sync.dma_start(t_[:], w[k, :, t, :])
                    wk[k][t] = t_
            bias_sb = wpool.tile([128, MT], f32, tag="bias")
            nc.sync.dma_start(bias_sb[:], bias[:, :])

            # x loads are chunked 4-way along positions (8-col tap halo per
            # chunk) so the 8 per-batch DMAs spread across the HWDGE queues:
            # the first matmul waits on one 528 KB chunk, not 4.2 MB on one
            # queue (the sim's 21.7 us startup PE stall).
            CW = LP // 4 + 6  # 1032: two 512-blocks + 8-tap halo

            def body(_iv=None):
                for b in range(BPC):
                    xb = [[None] * 4 for _ in range(KT)]
                    for c in range(4):
                        for k in range(KT):
                            t_ = xpool.tile(
                                [128, CW], f32r, name="xb", tag=f"x{k}c{c}"
                            )
                            # SWDGE keeps x prefetch off the HWDGE path that
                            # carries weight loads and output stores.
                            nc.gpsimd.dma_start(
                                t_[:],
                                xp[
                                    b,
                                    k * 128 : (k + 1) * 128,
                                    c * 1024 : c * 1024 + CW,
                                ],
                            )
                            xb[k][c] = t_
                    for m in range(MT):
                        pts = [
                            ppool.tile([128, NFREE], f32, name="pt", tag="pt")
                            for _ in range(NT)
                        ]
                        first = True
                        for k in range(KT):
                            for t in range(KW):
                                lhsT = wk[k][t][:, m * 128 : (m + 1) * 128]
                                last = k == KT - 1 and t == KW - 1
                                for j in range(NT):
                                    lo = (j % 2) * NFREE + t
                                    nc.tensor.matmul(
                                        pts[j][:, :],
                                        lhsT,
                                        xb[k][j // 2][:, lo : lo + NFREE],
                                        start=first,
                                        stop=last,
                                    )
                                first = False
                        # Alternate the psum->SBUF bias-add copy between ACT
                        # and DVE: halves the tail drain after the last
                        # matmul and frees PSUM banks sooner at m boundaries.
                        for j in range(NT):
                            ot = opool.tile([128, NFREE], f32, name="ot", tag="ot")
                            if j % 2 == 0:
                                nc.scalar.add(
                                    ot[:, :], pts[j][:, :], bias_sb[:, m : m + 1]
                                )
                            else:
                                nc.vector.tensor_scalar_add(
                                    ot[:, :], pts[j][:, :], bias_sb[:, m : m + 1]
                                )
                            nc.sync.dma_start(
                                out[
                                    b,
                                    m * 128 : (m + 1) * 128,
                                    j * NFREE : (j + 1) * NFREE,
                                ],
                                ot[:, :],
                            )

            # This container's walrus cannot codegen the For_i loop
            # machinery ("ISA wrong length" on the branch instructions),
            # so timing reps are software-unrolled instead.
            for _ in range(reps):
                body()

            if timing:
                dt_sb = opool.tile([128, 1], f32, name="dt_sb", tag="dt")
                nc.vector.memset(dt_sb[:, :], 0.0)
                nc.sync.dma_start(done[:, :], dt_sb[0:1, :])

    _split_multi_waits(nc)
    return nc


def kernel(x=None, weights=None, bias=None):
    from concourse.bass_utils import run_bass_kernel_spmd

    x = np.ascontiguousarray(np.asarray(x), dtype=np.float32)
    W = np.ascontiguousarray(np.asarray(weights), dtype=np.float32)
    bv = np.asarray(bias, dtype=np.float32)

    xpad = np.zeros((B, CIN, LP), np.float32)
    xpad[:, :, PAD : PAD + L] = x
    # w_arr[k, c, t, o] = W[o, k*128+c, t]  -> lhsT tiles are contiguous slices
    w_arr = np.ascontiguousarray(W.transpose(1, 2, 0).reshape(KT, 128, KW, COUT))
    bias_r = np.ascontiguousarray(bv.reshape(MT, 128).T)

    nc = _CACHE.get("nc")
    if nc is None:
        nc = _CACHE["nc"] = _build()

    in_maps = [
        {
            "xp": np.ascontiguousarray(xpad[c * BPC : (c + 1) * BPC]),
            "w": w_arr,
            "bias": bias_r,
        }
        for c in range(NCORES)
    ]
    # NTFF profiling needs an axon hook this container lacks; make sure a
    # stray BASS_TRACE in the environment cannot route us into that path.
    os.environ["BASS_NEVER_TRACE"] = "1"
    res = run_bass_kernel_spmd(nc, in_maps, core_ids=list(range(NCORES)))
    kernel.last_results = res
    results = res.results
    return np.concatenate([results[c]["out"] for c in range(NCORES)], axis=0)


kernel.last_results = None



# revision 3
# speedup vs baseline: 2.7695x; 2.7695x over previous
"""Conv1d  x[32,256,4096] * W[512,256,9] + b  (stride 1, pad 4)  -> [32,512,4096]
on 8 TRN2 NeuronCores, data-parallel over the batch dim (4 batches/core).

v2: bf16 operands (PE still 1 cycle/row, DMA and SBUF halved, rel err
~1e-3 vs the 2e-2 gate), and the per-m-block PSUM footprint split into
two 4-bank halves so the PSUM->SBUF bias-add drains overlap the next
half's matmuls instead of stalling the PE at m-block boundaries.

Matmul stream keeps the measured-good interleave: consecutive MMs hit
different PSUM banks (j innermost), each bank accumulating 18-deep
(2 k-tiles x 9 taps).
"""

import os

import numpy as np

B, CIN, COUT, KW, L, PAD = 32, 256, 512, 9, 4096, 4
NCORES = 8
BPC = B // NCORES  # batches per core
LP = L + 2 * PAD  # padded length
KT = CIN // 128  # contraction k-tiles
MT = COUT // 128  # output-channel tiles
NFREE = 512  # matmul moving free dim
NT = L // NFREE  # output-position tiles (8)
NH = NT // 4  # 4-bank halves per m-block (2)

_CACHE = {}

try:
    import ml_dtypes

    W_NP_DTYPE = ml_dtypes.bfloat16  # timing harness casts the w input
except ImportError:  # pragma: no cover
    W_NP_DTYPE = None

X_DGE = "hwdge"  # "swdge" (gpsimd) or "hwdge" (sync) for x prefetch
# measured: hwdge x-loads 527us/body vs swdge 634us (SWDGE Q7 emission +
# descriptor-ring port contention costs ~107us/body here)
OUT_DGE = "sp"  # "sp" (sync/SP HWDGE ring) or "act" (scalar/ACT HWDGE ring)
XCHUNKS = 4  # x chunks per (batch, k-tile): 4 x 264KB or 2 x 514KB
OUT_BF16 = False  # store conv output as bf16, upcast to f32 on host


def _split_multi_waits(nc, max_waits=1):
    # This container's walrus accepts at most one sync wait per instruction;
    # TileContext's tail drain carries several.  Hoist the excess onto
    # same-engine EventSemaphore instructions inserted just before it.
    import concourse.mybir as mybir

    for fn in nc.m.functions:
        for bb in fn.blocks:
            new_list = []
            changed = False
            for ins in bb.instructions:
                si = ins.sync_info
                if si is not None and si.on_wait and len(si.on_wait) > max_waits:
                    waits = list(si.on_wait)
                    hoist, keep = waits[:-max_waits], waits[-max_waits:]
                    for j, w in enumerate(hoist):
                        ev = mybir.InstEventSemaphore(
                            name=f"{ins.name}_wsplit{j}",
                            engine=ins.engine,
                            ins=[],
                            outs=[],
                            sync_info=mybir.SyncInfo(on_wait=[w], on_update=[]),
                        )
                        new_list.append(ev)
                    ins.sync_info = mybir.SyncInfo(
                        on_wait=keep, on_update=list(si.on_update)
                    )
                    changed = True
                new_list.append(ins)
            if changed:
                bb.instructions = new_list


def _build(reps=1, timing=False):
    """Build the per-core Bass module.

    timing=True makes the big I/O tensors Internal DRAM (nothing shipped
    through the axon tunnel) and software-unrolls `reps` copies of the
    body (this container's walrus cannot codegen For_i loop machinery)."""
    import concourse.bass as bass
    import concourse.mybir as mybir
    import concourse.tile as tile

    f32 = mybir.dt.float32
    bf16 = mybir.dt.bfloat16

    nc = bass.Bass()
    big_kind = "Internal" if timing else "ExternalInput"
    xp = nc.dram_tensor("xp", [BPC, CIN, LP], bf16, kind=big_kind)
    w = nc.dram_tensor("w", [KT, 128, KW, COUT], bf16, kind="ExternalInput")
    bias = nc.dram_tensor("bias", [128, MT], f32, kind="ExternalInput")
    out_dt = bf16 if OUT_BF16 else f32
    out = nc.dram_tensor(
        "out", [BPC, COUT, L], out_dt, kind="Internal" if timing else "ExternalOutput"
    )
    done = (
        nc.dram_tensor("done", [1, 1], f32, kind="ExternalOutput") if timing else None
    )

    # x chunking: XCHUNKS chunks of LP/XCHUNKS positions (+8 halo) per
    # (batch, k-tile)
    NCH = XCHUNKS
    CPOS = L // NCH  # positions per chunk (1024 or 2048)
    CW = CPOS + 8  # + 8-tap halo

    with tile.TileContext(nc) as tc:
        with (
            tc.tile_pool(name="wpool", bufs=1) as wpool,
            tc.tile_pool(name="xpool", bufs=1) as xpool,
            tc.tile_pool(name="opool", bufs=8) as opool,
            tc.tile_pool(name="psum", bufs=1, space="PSUM") as ppool,
        ):
            # One weight tile per (k-tile, tap), tap-0 first (gates first MMs).
            wk = [[None] * KW for _ in range(KT)]
            for t in range(KW):
                for k in range(KT):
                    t_ = wpool.tile([128, COUT], bf16, name="wkt", tag=f"w{k}_{t}")
                    nc.sync.dma_start(t_[:], w[k, :, t, :])
                    wk[k][t] = t_
            bias_sb = wpool.tile([128, MT], f32, name="bias_sb", tag="bias")
            nc.sync.dma_start(bias_sb[:], bias[:, :])

            # 8 PSUM banks: two 4-bank halves.
            pts = [
                ppool.tile([128, NFREE], f32, name=f"pt{j}", tag=f"pt{j}")
                for j in range(8)
            ]

            def load_x(b, tag_sfx):
                xb = [[None] * NCH for _ in range(KT)]
                for c in range(NCH):
                    for k in range(KT):
                        t_ = xpool.tile(
                            [128, CW], bf16, name="xb", tag=f"x{k}c{c}{tag_sfx}"
                        )
                        dge = nc.gpsimd if X_DGE == "swdge" else nc.sync
                        dge.dma_start(
                            t_[:],
                            xp[
                                b,
                                k * 128 : (k + 1) * 128,
                                c * CPOS : c * CPOS + CW,
                            ],
                        )
                        xb[k][c] = t_
                return xb

            def body(_iv=None):
                xb_next = load_x(0, "a")
                for b in range(BPC):
                    xb = xb_next
                    if b + 1 < BPC:
                        xb_next = load_x(b + 1, "b" if b % 2 == 0 else "a")
                    for m in range(MT):
                        for h in range(NH):
                            bank0 = (h % 2) * 4
                            first = True
                            for k in range(KT):
                                for t in range(KW):
                                    lhsT = wk[k][t][:, m * 128 : (m + 1) * 128]
                                    last = k == KT - 1 and t == KW - 1
                                    for j in range(4):
                                        # global position tile index
                                        jj = h * 4 + j
                                        tpc = CPOS // NFREE
                                        lo = (jj % tpc) * NFREE + t
                                        nc.tensor.matmul(
                                            pts[bank0 + j][:, :],
                                            lhsT,
                                            xb[k][jj // tpc][:, lo : lo + NFREE],
                                            start=first,
                                            stop=last,
                                        )
                                    first = False
                            # Drain this half: bias-add PSUM->SBUF split
                            # over ACT and DVE, store via HWDGE.
                            for j in range(4):
                                jj = h * 4 + j
                                ot = opool.tile(
                                    [128, NFREE], out_dt, name="ot", tag="ot"
                                )
                                if j % 2 == 0:
                                    nc.scalar.add(
                                        ot[:, :],
                                        pts[bank0 + j][:, :],
                                        bias_sb[:, m : m + 1],
                                    )
                                else:
                                    nc.vector.tensor_scalar_add(
                                        ot[:, :],
                                        pts[bank0 + j][:, :],
                                        bias_sb[:, m : m + 1],
                                    )
                                odge = nc.sync if OUT_DGE == "sp" else nc.scalar
                                odge.dma_start(
                                    out[
                                        b,
                                        m * 128 : (m + 1) * 128,
                                        jj * NFREE : (jj + 1) * NFREE,
                                    ],
                                    ot[:, :],
                                )

            for _ in range(reps):
                body()

            if timing:
                dt_sb = opool.tile([128, 1], f32, name="dt_sb", tag="dt")
                nc.scalar.copy(dt_sb[:, :], pts[0][:, 0:1])
                nc.sync.dma_start(done[:, :], dt_sb[0:1, :])

    _split_multi_waits(nc)
    return nc


def kernel(x=None, weights=None, bias=None):
    import ml_dtypes
    from concourse.bass_utils import run_bass_kernel_spmd

    x = np.ascontiguousarray(np.asarray(x), dtype=np.float32)
    W = np.ascontiguousarray(np.asarray(weights), dtype=np.float32)
    bv = np.asarray(bias, dtype=np.float32)

    bf = ml_dtypes.bfloat16
    xpad = np.zeros((B, CIN, LP), bf)
    xpad[:, :, PAD : PAD + L] = x.astype(bf)
    # w_arr[k, c, t, o] = W[o, k*128+c, t]  -> lhsT tiles are contiguous slices
    w_arr = np.ascontiguousarray(
        W.transpose(1, 2, 0).reshape(KT, 128, KW, COUT).astype(bf)
    )
    bias_r = np.ascontiguousarray(bv.reshape(MT, 128).T)

    nc = _CACHE.get("nc")
    if nc is None:
        nc = _CACHE["nc"] = _build()

    in_maps = [
        {
            "xp": np.ascontiguousarray(xpad[c * BPC : (c + 1) * BPC]),
            "w": w_arr,
            "bias": bias_r,
        }
        for c in range(NCORES)
    ]
    # NTFF profiling needs an axon hook this container lacks; make sure a
    # stray BASS_TRACE in the environment cannot route us into that path.
    os.environ["BASS_NEVER_TRACE"] = "1"
    res = run_bass_kernel_spmd(nc, in_maps, core_ids=list(range(NCORES)))
    kernel.last_results = res
    results = res.results
    full = np.concatenate([results[c]["out"] for c in range(NCORES)], axis=0)
    return full.astype(np.float32, copy=False)


kernel.last_results = None
